# revision 38
# baseline (speedup 1.0000x reference)
"""Causal attention (B=8, N=4096, D=64) on 8 trn2 NeuronCores.

Sharding: batch b -> core b (data parallel, no cross-core comms).

Per-core kernel (flash-attention style, fully transposed dataflow):
  inputs (host pre-layouts):  qT [64, N], kT [64, N]   (d on partitions),
                              v_aug [128, N/128, 65]   (k-tiled; col 64 = 1.0;
                                                        padding-masked rows = 0)
  for each q-block (512 wide):
    for each causal k-tile PAIR (2 x 128 wide):
      logitsT[k, q]  = matmul(lhsT=kT_t  [64,128], rhs=qT_blk[64,512])  (PSUM)
      logitsT'[k, q] = matmul(lhsT=kT_t1 [64,128], rhs=qT_blk[64,512])  (PSUM)
      expT = exp(logitsT_pair * 1/sqrt(d))     one ACT op over [128,1024] ->SBUF
      if diagonal: expT half *= causal 0/1 mask tile                    (DVE)
      outT[d,q] (+)= matmul(lhsT=v_aug[128,65], rhs=expT_half[128,512]) (PSUM)
        -- v_aug col 64 is 1.0 => outT row 64 = softmax denominators
  per q-block: r = 1/outT[64]; bc = ones[64] (x) r (PE outer product);
               out = outT[0:64] * bc (DVE); DMA out -> outT_dram[:, q-block]
  host transposes outT_dram [64, N] back to [N, 64] at gather time.

Padding mask: host zeroes masked k rows of v_aug (incl. the ones column), so
masked keys contribute nothing to numerator or denominator -- exactly
equivalent to -inf logits.

All matmuls use float32r (full-rate fp32 on the PE at moving-dim >= 256).
"""

import os
from contextlib import ExitStack

import numpy as np

B, N, D = 8, 4096, 64
QBLK = 512
KTILE = 128

LAST_RESULTS = None
_NC_CACHE = {}


def build(n=N, d=D, qblk=QBLK, ktile=KTILE, lg_bufs=3, acc_bufs=2, pb_bufs=6,
          op_dt="float16", epi_depth=1):
    import concourse.bass as bass
    import concourse.mybir as mybir
    import concourse.tile as tile
    from concourse import bacc

    f32 = mybir.dt.float32
    f32r = mybir.dt.float32r
    opd = getattr(mybir.dt, op_dt)   # matmul operand dtype (fp16 or f32r)
    nt = n // ktile          # number of k-tiles
    nqb = n // qblk          # number of q-blocks
    tpq = qblk // ktile      # k-tiles per q-block (diagonal span)
    assert tpq % 2 == 0

    nc = bacc.Bacc("TRN2", target_bir_lowering=False, debug=False,
                   enable_asserts=False)

    qk_d = nc.dram_tensor("qk", (d, nqb, 2, qblk), opd,
                          kind="ExternalInput").ap()
    v_d = nc.dram_tensor("v_aug", (128, nt, d + 1), opd,
                         kind="ExternalInput").ap()
    mk_d = nc.dram_tensor("cmasks", (128, tpq, qblk), opd,
                          kind="ExternalInput").ap()
    oT_d = nc.dram_tensor("outT", (d, n), f32, kind="ExternalOutput").ap()
    rs_d = nc.dram_tensor("rs_scratch", (nqb, qblk), f32,
                          kind="Internal").ap()

    scale = 1.0 / float(np.sqrt(d))

    with tile.TileContext(nc) as tc:
        with ExitStack() as ctx:
            singles = ctx.enter_context(tc.tile_pool(name="singles", bufs=1))
            pb_pool = ctx.enter_context(tc.tile_pool(name="pb", bufs=pb_bufs))
            small = ctx.enter_context(tc.tile_pool(name="small", bufs=2))
            ob_pool = ctx.enter_context(tc.tile_pool(name="ob", bufs=3))
            lg_pool = ctx.enter_context(
                tc.tile_pool(name="lg", bufs=lg_bufs, space="PSUM"))
            acc_pool = ctx.enter_context(
                tc.tile_pool(name="acc", bufs=acc_bufs, space="PSUM"))

            # --- resident inputs -------------------------------------------
            qk_sb = singles.tile([d, nqb, 2, qblk], opd)
            v_sb = singles.tile([128, nt, d + 1], opd)
            mk_sb = singles.tile([128, tpq, qblk], opd)

            # Two q-block streams run interleaved; longest-first order keeps
            # both busy to the end (balanced makespan). DMA chunks in
            # first-use order: the largest block's qT, then small blocks'
            # chunks (which double as the early kT tiles every stream needs).
            lpt = sorted(range(nqb), key=lambda c: -c)   # [7,6,5,...,0]
            dma_order = []
            for i, c in enumerate(lpt):
                dma_order.append(c)
                lo = lpt[nqb - 1 - i]
                if lo not in dma_order:
                    dma_order.append(lo)
            vdma = [None] * nqb
            qkdma = [None] * nqb
            mkdma_done = False
            for idx, c in enumerate(dma_order):
                qkdma[c] = nc.sync.dma_start(
                    out=qk_sb[:, c, :, :], in_=qk_d[:, c, :, :])
                vs, ve = c * tpq, min(nt, (c + 1) * tpq)
                vdma[c] = nc.sync.dma_start(
                    out=v_sb[:, vs:ve, :], in_=v_d[:, vs:ve, :])
                if idx == 1 and not mkdma_done:
                    nc.sync.dma_start(out=mk_sb, in_=mk_d)
                    mkdma_done = True
            if not mkdma_done:
                nc.sync.dma_start(out=mk_sb, in_=mk_d)

            masks = [mk_sb[:, j, :] for j in range(tpq)]

            def kT_ap(t):
                c, r = divmod(t, tpq)
                return qk_sb[:, c, 0, r * ktile:(r + 1) * ktile]

            # --- main loop -------------------------------------------------
            def epilogue(acc, qs, qb):
                # normalize: out = outT[0:64] / sums (sums = row d of acc).
                # The per-q reciprocal is broadcast across partitions with a
                # DRAM round-trip (partition-step-0 reads are DRAM-only), so
                # the whole epilogue stays off the PE.
                rsum = small.tile([1, qblk], f32, name="rsum")
                nc.vector.reciprocal(rsum, acc[d:d + 1, :])
                nc.sync.dma_start(out=rs_d[qb:qb + 1, :], in_=rsum)
                rb = ob_pool.tile([d, qblk], f32, name="rb")
                rs_slice = rs_d[qb:qb + 1, :]
                brd = bass.AP(tensor=rs_slice.tensor, offset=rs_slice.offset,
                              ap=[[0, d], list(rs_slice.ap[-1])])
                nc.sync.dma_start(out=rb, in_=brd)
                ob = ob_pool.tile([d, qblk], f32, name="ob")
                nc.vector.tensor_mul(ob, acc[0:d, :], rb)
                nc.sync.dma_start(out=oT_d[:, qs:qs + qblk], in_=ob)

            # Per global pair p: emit MM1s(p) + exp(p) [+ masks], then the
            # MM2s of pair p-1. This orders the PE stream as
            # [... MM1a(p) MM1b(p) MM2a(p-1) MM2b(p-1) ...] so the PE fills
            # the exp(p-1) latency with pair p's MM1s instead of stalling.
            mm2_q = []     # deferred MM2 emission: (acc, pb, t0, qb, tlast)

            def flush_mm2():
                acc_, pb_, t0_, qb_, tlast_ = mm2_q.pop(0)
                for h in range(2):
                    t = t0_ + h
                    nc.tensor.matmul(
                        acc_,
                        lhsT=v_sb[:, t, :],
                        rhs=pb_[:, h, :],
                        start=(t == 0), stop=(t == tlast_),
                    )
                if t0_ + 1 == tlast_:  # last pair of q-block: normalize now
                    epilogue(acc_, qb_ * qblk, qb_)

            def emit_pair(st, p):
                # one (MM1 x2 -> exp -> masks) group; MM2s deferred
                qb = st["qb"]
                t0 = 2 * p
                lg = lg_pool.tile([128, 2, qblk], f32, name="lg")
                pb = pb_pool.tile([128, 2, qblk], opd, name="pb")
                for h in range(2):
                    t = t0 + h
                    nc.tensor.matmul(
                        lg[:, h, :],
                        lhsT=kT_ap(t),
                        rhs=st["q_sl"],
                        start=True, stop=True,
                    )
                nc.scalar.activation(
                    pb, lg, mybir.ActivationFunctionType.Exp,
                    scale=scale,
                )
                for h in range(2):
                    j = t0 + h - tpq * qb
                    if j >= 0:
                        nc.vector.tensor_mul(
                            pb[:, h, :], pb[:, h, :], masks[j])
                mm2_q.append((st["acc"], pb, t0, qb, st["tlast"]))
                if len(mm2_q) >= 2:
                    flush_mm2()

            def new_stream(qb):
                # lazy acc allocation keeps at most 2 acc tiles live
                acc = acc_pool.tile([d + 1, qblk], f32, name="acc", tag="acc")
                npairs = (tpq * qb + tpq) // 2
                return {"qb": qb, "acc": acc, "q_sl": qk_sb[:, qb, 1, :],
                        "npairs": npairs, "tlast": 2 * npairs - 1}

            # two stream slots, each popping the next q-block (longest
            # first) when its current one finishes -- no phase barriers
            queue = list(lpt)
            slots = [None, None]
            prog = [0, 0]
            while True:
                # pick the emptier slot (fewer emitted pairs) that has work
                order = sorted(range(2), key=lambda s: prog[s])
                advanced = False
                for s in order:
                    if slots[s] is None or slots[s]["done"]:
                        if queue:
                            qb = queue.pop(0)
                            st = new_stream(qb)
                            st["done"] = False
                            st["p"] = 0
                            slots[s] = st
                        else:
                            continue
                    st = slots[s]
                    emit_pair(st, st["p"])
                    st["p"] += 1
                    prog[s] += 1
                    if st["p"] >= st["npairs"]:
                        st["done"] = True
                    advanced = True
                    break
                if not advanced:
                    break
            while mm2_q:
                flush_mm2()

    nc.compile()
    return nc


def _get_nc(key="main", **kw):
    if key not in _NC_CACHE:
        _NC_CACHE[key] = build(**kw)
    return _NC_CACHE[key]


def _prep_core_inputs(q, k, v, attn_mask, b, n=N, d=D, ktile=KTILE,
                      qblk=QBLK, op_dt="float16"):
    npdt = np.float16 if op_dt == "float16" else np.float32
    nt = n // ktile
    nqb = n // qblk
    qT = q[b].T.astype(npdt)          # [d, n]
    kT = k[b].T.astype(npdt)
    qk = np.empty((d, nqb, 2, qblk), dtype=npdt)
    qk[:, :, 0, :] = kT.reshape(d, nqb, qblk)
    qk[:, :, 1, :] = qT.reshape(d, nqb, qblk)
    v_aug = np.ones((n, d + 1), dtype=np.float32)
    v_aug[:, :d] = v[b]
    v_aug *= (attn_mask[b] != 0).astype(np.float32)[:, None]
    v_aug = np.ascontiguousarray(
        v_aug.reshape(nt, ktile, d + 1).transpose(1, 0, 2)).astype(npdt)
    tpq = qblk // ktile
    # causal 0/1 mask per diagonal alignment j: keep where q >= k + 128*j
    y = np.arange(qblk)[None, None, :]
    x = np.arange(ktile)[:, None, None]
    jj = np.arange(tpq)[None, :, None]
    cmasks = (y - x - ktile * jj >= 0).astype(npdt)
    return {"qk": qk, "v_aug": v_aug, "cmasks": cmasks}


def kernel(q, k, v, attn_mask):
    global LAST_RESULTS
    q = np.asarray(q, dtype=np.float32)
    k = np.asarray(k, dtype=np.float32)
    v = np.asarray(v, dtype=np.float32)
    attn_mask = np.asarray(attn_mask)

    from concourse.bass_utils import run_bass_kernel_spmd

    nc = _get_nc()
    in_maps = [_prep_core_inputs(q, k, v, attn_mask, b) for b in range(B)]
    trace = bool(os.environ.get("BASS_TRACE"))
    LAST_RESULTS = run_bass_kernel_spmd(
        nc, in_maps, core_ids=list(range(B)), trace=trace)

    out = np.empty((B, N, D), dtype=np.float32)
    for b in range(B):
        out[b] = LAST_RESULTS.results[b]["outT"].T
    return out


# revision 40
# speedup vs baseline: 1.4280x; 1.4280x over previous
"""Causal attention (B=8, N=4096, D=64) on 8 trn2 NeuronCores.

Sharding: batch b -> core b (data parallel, no cross-core comms).

Per-core kernel (flash-attention style, fully transposed dataflow):
  inputs (host pre-layouts):  qT [64, N], kT [64, N]   (d on partitions),
                              v_aug [128, N/128, 65]   (k-tiled; col 64 = 1.0;
                                                        padding-masked rows = 0)
  for each q-block (512 wide):
    for each causal k-tile PAIR (2 x 128 wide):
      logitsT[k, q]  = matmul(lhsT=kT_t  [64,128], rhs=qT_blk[64,512])  (PSUM)
      logitsT'[k, q] = matmul(lhsT=kT_t1 [64,128], rhs=qT_blk[64,512])  (PSUM)
      expT = exp(logitsT_pair * 1/sqrt(d))     one ACT op over [128,1024] ->SBUF
      if diagonal: expT half *= causal 0/1 mask tile                    (DVE)
      outT[d,q] (+)= matmul(lhsT=v_aug[128,65], rhs=expT_half[128,512]) (PSUM)
        -- v_aug col 64 is 1.0 => outT row 64 = softmax denominators
  per q-block: r = 1/outT[64]; bc = ones[64] (x) r (PE outer product);
               out = outT[0:64] * bc (DVE); DMA out -> outT_dram[:, q-block]
  host transposes outT_dram [64, N] back to [N, 64] at gather time.

Padding mask: host zeroes masked k rows of v_aug (incl. the ones column), so
masked keys contribute nothing to numerator or denominator -- exactly
equivalent to -inf logits.

All matmuls use float32r (full-rate fp32 on the PE at moving-dim >= 256).
"""

import os
from contextlib import ExitStack

import numpy as np

B, N, D = 8, 4096, 64
QBLK = 512
KTILE = 128

LAST_RESULTS = None
_NC_CACHE = {}


def build(n=N, d=D, qblk=QBLK, ktile=KTILE, lg_bufs=3, acc_bufs=2, pb_bufs=6,
          op_dt="float16", epi_depth=1):
    import concourse.bass as bass
    import concourse.mybir as mybir
    import concourse.tile as tile
    from concourse import bacc

    f32 = mybir.dt.float32
    f32r = mybir.dt.float32r
    opd = getattr(mybir.dt, op_dt)   # matmul operand dtype (fp16 or f32r)
    nt = n // ktile          # number of k-tiles
    nqb = n // qblk          # number of q-blocks
    tpq = qblk // ktile      # k-tiles per q-block (diagonal span)
    assert tpq % 2 == 0

    nc = bacc.Bacc("TRN2", target_bir_lowering=False, debug=False,
                   enable_asserts=False)

    qk_d = nc.dram_tensor("qk", (d, nqb, 2, qblk), opd,
                          kind="ExternalInput").ap()
    v_d = nc.dram_tensor("v_aug", (128, nt, d + 1), opd,
                         kind="ExternalInput").ap()
    mk_d = nc.dram_tensor("cmasks", (128, tpq, qblk), opd,
                          kind="ExternalInput").ap()
    oT_d = nc.dram_tensor("outT", (d, n), f32, kind="ExternalOutput").ap()
    rs_d = nc.dram_tensor("rs_scratch", (nqb, qblk), f32,
                          kind="Internal").ap()

    scale = 1.0 / float(np.sqrt(d))

    with tile.TileContext(nc) as tc:
        with ExitStack() as ctx:
            singles = ctx.enter_context(tc.tile_pool(name="singles", bufs=1))
            pb_pool = ctx.enter_context(tc.tile_pool(name="pb", bufs=pb_bufs))
            small = ctx.enter_context(tc.tile_pool(name="small", bufs=2))
            ob_pool = ctx.enter_context(tc.tile_pool(name="ob", bufs=3))
            lg_pool = ctx.enter_context(
                tc.tile_pool(name="lg", bufs=lg_bufs, space="PSUM"))
            acc_pool = ctx.enter_context(
                tc.tile_pool(name="acc", bufs=acc_bufs, space="PSUM"))

            # --- resident inputs -------------------------------------------
            qk_sb = singles.tile([d, nqb, 2, qblk], opd)
            v_sb = singles.tile([128, nt, d + 1], opd)
            mk_sb = singles.tile([128, tpq, qblk], opd)

            # few big DMAs (the ~650ns per-DMA issue on the sync queue is
            # serial and delayed the first matmul by ~5us when chunked)
            gchunk = max(1, nqb // 4)
            for c in range(0, nqb, gchunk):
                ce = min(nqb, c + gchunk)
                nc.sync.dma_start(out=qk_sb[:, c:ce, :, :],
                                  in_=qk_d[:, c:ce, :, :])
                if c == 0:
                    nc.sync.dma_start(out=mk_sb, in_=mk_d)
                vs, ve = c * tpq, min(nt, ce * tpq)
                nc.sync.dma_start(out=v_sb[:, vs:ve, :], in_=v_d[:, vs:ve, :])

            masks = [mk_sb[:, j, :] for j in range(tpq)]

            def kT_ap(t):
                c, r = divmod(t, tpq)
                return qk_sb[:, c, 0, r * ktile:(r + 1) * ktile]

            # --- main loop -------------------------------------------------
            def epilogue(acc, qs, qb):
                # normalize: out = outT[0:64] / sums (sums = row d of acc).
                # The per-q reciprocal is broadcast across partitions with a
                # DRAM round-trip (partition-step-0 reads are DRAM-only), so
                # the whole epilogue stays off the PE.
                rsum = small.tile([1, qblk], f32, name="rsum")
                nc.vector.reciprocal(rsum, acc[d:d + 1, :])
                nc.sync.dma_start(out=rs_d[qb:qb + 1, :], in_=rsum)
                rb = ob_pool.tile([d, qblk], f32, name="rb")
                rs_slice = rs_d[qb:qb + 1, :]
                brd = bass.AP(tensor=rs_slice.tensor, offset=rs_slice.offset,
                              ap=[[0, d], list(rs_slice.ap[-1])])
                nc.sync.dma_start(out=rb, in_=brd)
                ob = ob_pool.tile([d, qblk], f32, name="ob")
                nc.vector.tensor_mul(ob, acc[0:d, :], rb)
                nc.sync.dma_start(out=oT_d[:, qs:qs + qblk], in_=ob)

            # Per global pair p: emit MM1s(p) + exp(p) [+ masks], then the
            # MM2s of pair p-1. This orders the PE stream as
            # [... MM1a(p) MM1b(p) MM2a(p-1) MM2b(p-1) ...] so the PE fills
            # the exp(p-1) latency with pair p's MM1s instead of stalling.
            mm2_q = []     # deferred MM2 emission: (acc, pb, t0, qb, tlast)

            def flush_mm2():
                acc_, pb_, t0_, qb_, tlast_ = mm2_q.pop(0)
                for h in range(2):
                    t = t0_ + h
                    nc.tensor.matmul(
                        acc_,
                        lhsT=v_sb[:, t, :],
                        rhs=pb_[:, h, :],
                        start=(t == 0), stop=(t == tlast_),
                    )
                if t0_ + 1 == tlast_:  # last pair of q-block: normalize now
                    epilogue(acc_, qb_ * qblk, qb_)

            def emit_pair(st, p):
                # one (MM1 x2 -> exp -> masks) group; MM2s deferred
                qb = st["qb"]
                t0 = 2 * p
                lg = lg_pool.tile([128, 2, qblk], f32, name="lg")
                pb = pb_pool.tile([128, 2, qblk], opd, name="pb")
                for h in range(2):
                    t = t0 + h
                    nc.tensor.matmul(
                        lg[:, h, :],
                        lhsT=kT_ap(t),
                        rhs=st["q_sl"],
                        start=True, stop=True,
                    )
                nc.scalar.activation(
                    pb, lg, mybir.ActivationFunctionType.Exp,
                    scale=scale,
                )
                for h in range(2):
                    j = t0 + h - tpq * qb
                    if j >= 0:
                        nc.vector.tensor_mul(
                            pb[:, h, :], pb[:, h, :], masks[j])
                mm2_q.append((st["acc"], pb, t0, qb, st["tlast"]))
                if len(mm2_q) >= 2:
                    flush_mm2()

            def new_stream(qb):
                # lazy acc allocation keeps at most 2 acc tiles live
                acc = acc_pool.tile([d + 1, qblk], f32, name="acc", tag="acc")
                npairs = (tpq * qb + tpq) // 2
                return {"qb": qb, "acc": acc, "q_sl": qk_sb[:, qb, 1, :],
                        "npairs": npairs, "tlast": 2 * npairs - 1}

            for qb in range(nqb):
                st = new_stream(qb)
                for p in range(st["npairs"]):
                    emit_pair(st, p)
            while mm2_q:
                flush_mm2()

    nc.compile()
    return nc


def _get_nc(key="main", **kw):
    if key not in _NC_CACHE:
        _NC_CACHE[key] = build(**kw)
    return _NC_CACHE[key]


def _prep_core_inputs(q, k, v, attn_mask, b, n=N, d=D, ktile=KTILE,
                      qblk=QBLK, op_dt="float16"):
    npdt = np.float16 if op_dt == "float16" else np.float32
    nt = n // ktile
    nqb = n // qblk
    qT = q[b].T.astype(npdt)          # [d, n]
    kT = k[b].T.astype(npdt)
    qk = np.empty((d, nqb, 2, qblk), dtype=npdt)
    qk[:, :, 0, :] = kT.reshape(d, nqb, qblk)
    qk[:, :, 1, :] = qT.reshape(d, nqb, qblk)
    v_aug = np.ones((n, d + 1), dtype=np.float32)
    v_aug[:, :d] = v[b]
    v_aug *= (attn_mask[b] != 0).astype(np.float32)[:, None]
    v_aug = np.ascontiguousarray(
        v_aug.reshape(nt, ktile, d + 1).transpose(1, 0, 2)).astype(npdt)
    tpq = qblk // ktile
    # causal 0/1 mask per diagonal alignment j: keep where q >= k + 128*j
    y = np.arange(qblk)[None, None, :]
    x = np.arange(ktile)[:, None, None]
    jj = np.arange(tpq)[None, :, None]
    cmasks = (y - x - ktile * jj >= 0).astype(npdt)
    return {"qk": qk, "v_aug": v_aug, "cmasks": cmasks}


def kernel(q, k, v, attn_mask):
    global LAST_RESULTS
    q = np.asarray(q, dtype=np.float32)
    k = np.asarray(k, dtype=np.float32)
    v = np.asarray(v, dtype=np.float32)
    attn_mask = np.asarray(attn_mask)

    from concourse.bass_utils import run_bass_kernel_spmd

    nc = _get_nc()
    in_maps = [_prep_core_inputs(q, k, v, attn_mask, b) for b in range(B)]
    trace = bool(os.environ.get("BASS_TRACE"))
    LAST_RESULTS = run_bass_kernel_spmd(
        nc, in_maps, core_ids=list(range(B)), trace=trace)

    out = np.empty((B, N, D), dtype=np.float32)
    for b in range(B):
        out[b] = LAST_RESULTS.results[b]["outT"].T
    return out


# revision 45
# speedup vs baseline: 1.5137x; 1.0600x over previous
"""Causal attention (B=8, N=4096, D=64) on 8 trn2 NeuronCores.

Sharding: batch b -> core b (data parallel, no cross-core comms).

Per-core kernel (flash-attention style, fully transposed dataflow):
  inputs (host pre-layouts):  qT [64, N], kT [64, N]   (d on partitions),
                              v_aug [128, N/128, 65]   (k-tiled; col 64 = 1.0;
                                                        padding-masked rows = 0)
  for each q-block (512 wide):
    for each causal k-tile PAIR (2 x 128 wide):
      logitsT[k, q]  = matmul(lhsT=kT_t  [64,128], rhs=qT_blk[64,512])  (PSUM)
      logitsT'[k, q] = matmul(lhsT=kT_t1 [64,128], rhs=qT_blk[64,512])  (PSUM)
      expT = exp(logitsT_pair * 1/sqrt(d))     one ACT op over [128,1024] ->SBUF
      if diagonal: expT half *= causal 0/1 mask tile                    (DVE)
      outT[d,q] (+)= matmul(lhsT=v_aug[128,65], rhs=expT_half[128,512]) (PSUM)
        -- v_aug col 64 is 1.0 => outT row 64 = softmax denominators
  per q-block: r = 1/outT[64]; bc = ones[64] (x) r (PE outer product);
               out = outT[0:64] * bc (DVE); DMA out -> outT_dram[:, q-block]
  host transposes outT_dram [64, N] back to [N, 64] at gather time.

Padding mask: host zeroes masked k rows of v_aug (incl. the ones column), so
masked keys contribute nothing to numerator or denominator -- exactly
equivalent to -inf logits.

All matmuls use float32r (full-rate fp32 on the PE at moving-dim >= 256).
"""

import os
from contextlib import ExitStack

import numpy as np

B, N, D = 8, 4096, 64
QBLK = 512
KTILE = 128

LAST_RESULTS = None
_NC_CACHE = {}


def build(n=N, d=D, qblk=QBLK, ktile=KTILE, lg_bufs=2, acc_bufs=2, pb_bufs=6,
          op_dt="float16", epi_depth=1):
    import concourse.bass as bass
    import concourse.mybir as mybir
    import concourse.tile as tile
    from concourse import bacc

    f32 = mybir.dt.float32
    f32r = mybir.dt.float32r
    opd = getattr(mybir.dt, op_dt)   # matmul operand dtype (fp16 or f32r)
    qblk = min(qblk, n)
    nt = n // ktile          # number of k-tiles
    nqb = n // qblk          # number of q-blocks
    tpq = qblk // ktile      # k-tiles per q-block (diagonal span)

    nc = bacc.Bacc("TRN2", target_bir_lowering=False, debug=False,
                   enable_asserts=False)

    qk_d = nc.dram_tensor("qk", (d, nqb, 2, qblk), opd,
                          kind="ExternalInput").ap()
    v_d = nc.dram_tensor("v_aug", (128, nt, d + 1), opd,
                         kind="ExternalInput").ap()
    mk_d = nc.dram_tensor("cmasks", (128, ktile), opd,
                          kind="ExternalInput").ap()
    oT_d = nc.dram_tensor("outT", (d, n), f32, kind="ExternalOutput").ap()
    rs_d = nc.dram_tensor("rs_scratch", (nqb, qblk), f32,
                          kind="Internal").ap()

    scale = 1.0 / float(np.sqrt(d))

    with tile.TileContext(nc) as tc:
        with ExitStack() as ctx:
            singles = ctx.enter_context(tc.tile_pool(name="singles", bufs=1))
            pb_pool = ctx.enter_context(tc.tile_pool(name="pb", bufs=pb_bufs))
            small = ctx.enter_context(tc.tile_pool(name="small", bufs=2))
            ob_pool = ctx.enter_context(tc.tile_pool(name="ob", bufs=3))
            lg_pool = ctx.enter_context(
                tc.tile_pool(name="lg", bufs=lg_bufs, space="PSUM"))
            acc_pool = ctx.enter_context(
                tc.tile_pool(name="acc", bufs=acc_bufs, space="PSUM"))

            # --- resident inputs -------------------------------------------
            qk_sb = singles.tile([d, nqb, 2, qblk], opd)
            v_sb = singles.tile([128, nt, d + 1], opd)
            mk_sb = singles.tile([128, ktile], opd)

            # few big DMAs (the ~650ns per-DMA issue on the sync queue is
            # serial and delayed the first matmul by ~5us when chunked)
            gchunk = max(1, nqb // 4)
            for c in range(0, nqb, gchunk):
                ce = min(nqb, c + gchunk)
                nc.sync.dma_start(out=qk_sb[:, c:ce, :, :],
                                  in_=qk_d[:, c:ce, :, :])
                if c == 0:
                    nc.sync.dma_start(out=mk_sb, in_=mk_d)
                vs, ve = c * tpq, min(nt, ce * tpq)
                nc.sync.dma_start(out=v_sb[:, vs:ve, :], in_=v_d[:, vs:ve, :])

            def kT_ap(t):
                c, r = divmod(t, tpq)
                return qk_sb[:, c, 0, r * ktile:(r + 1) * ktile]

            # --- main loop -------------------------------------------------
            def epilogue(acc, qs, qb):
                # normalize: out = outT[0:64] / sums (sums = row d of acc).
                # The per-q reciprocal is broadcast across partitions with a
                # DRAM round-trip (partition-step-0 reads are DRAM-only), so
                # the whole epilogue stays off the PE.
                rsum = small.tile([1, qblk], f32, name="rsum")
                nc.vector.reciprocal(rsum, acc[d:d + 1, :])
                nc.sync.dma_start(out=rs_d[qb:qb + 1, :], in_=rsum)
                rb = ob_pool.tile([d, qblk], f32, name="rb")
                rs_slice = rs_d[qb:qb + 1, :]
                brd = bass.AP(tensor=rs_slice.tensor, offset=rs_slice.offset,
                              ap=[[0, d], list(rs_slice.ap[-1])])
                nc.sync.dma_start(out=rb, in_=brd)
                ob = ob_pool.tile([d, qblk], f32, name="ob")
                nc.vector.tensor_mul(ob, acc[0:d, :], rb)
                nc.sync.dma_start(out=oT_d[:, qs:qs + qblk], in_=ob)

            # Per k-tile PAIR: emit MM1s + exp(s) [+ boundary masks], then
            # the deferred MM2s of the previous pair, so the PE stream
            # interleaves [... MM1s(p) MM2s(p-1) ...] and fills the exp
            # latency. Diagonal tiles (j = t - tpq*qb >= 0) read/write only
            # their live columns [128*j, qblk) in MM2 (and in MM1/exp when
            # that is free); the 128-wide boundary strip gets the
            # triangular 0/1 mask.
            mm2_q = []   # deferred MM2s: (acc, pb, t0, lows, qb, tlast)

            def flush_mm2():
                acc_, pb_, t0_, lows, qb_, tlast_ = mm2_q.pop(0)
                for h in range(2):
                    t = t0_ + h
                    nc.tensor.matmul(
                        acc_[:, lows[h]:],
                        lhsT=v_sb[:, t, :],
                        rhs=pb_[:, h, lows[h]:],
                        start=(t == 0), stop=(t == tlast_),
                    )
                if t0_ + 1 == tlast_:   # last pair: normalize this q-block
                    epilogue(acc_, qb_ * qblk, qb_)

            for qb in range(nqb):
                q_sl = qk_sb[:, qb, 1, :]
                acc = acc_pool.tile([d + 1, qblk], f32, name="acc", tag="acc")
                npairs = (tpq * qb + tpq) // 2
                tlast = 2 * npairs - 1
                for p in range(npairs):
                    t0 = 2 * p
                    js = [t0 - tpq * qb, t0 + 1 - tpq * qb]
                    lows = [max(0, j) * ktile for j in js]
                    # only trim MM1/exp when both halves can be trimmed
                    # without splitting the exp unprofitably (j >= 2)
                    trim = js[0] >= 2
                    lg = lg_pool.tile([128, 2, qblk], f32, name="lg")
                    pb = pb_pool.tile([128, 2, qblk], opd, name="pb")
                    for h in range(2):
                        mlo = lows[h] if trim else 0
                        nc.tensor.matmul(
                            lg[:, h, mlo:],
                            lhsT=kT_ap(t0 + h),
                            rhs=q_sl[:, mlo:],
                            start=True, stop=True,
                        )
                    if trim:
                        for h in range(2):
                            nc.scalar.activation(
                                pb[:, h, lows[h]:], lg[:, h, lows[h]:],
                                mybir.ActivationFunctionType.Exp, scale=scale)
                    else:
                        nc.scalar.activation(
                            pb, lg, mybir.ActivationFunctionType.Exp,
                            scale=scale)
                    for h in range(2):
                        if js[h] >= 0:
                            sl = slice(lows[h], lows[h] + ktile)
                            nc.vector.tensor_mul(
                                pb[:, h, sl], pb[:, h, sl], mk_sb)
                    mm2_q.append((acc, pb, t0, lows, qb, tlast))
                    if len(mm2_q) >= 2:
                        flush_mm2()
            while mm2_q:
                flush_mm2()

    nc.compile()
    return nc


def _get_nc(key="main", **kw):
    if key not in _NC_CACHE:
        _NC_CACHE[key] = build(**kw)
    return _NC_CACHE[key]


def _prep_core_inputs(q, k, v, attn_mask, b, n=N, d=D, ktile=KTILE,
                      qblk=QBLK, op_dt="float16"):
    npdt = np.float16 if op_dt == "float16" else np.float32
    qblk = min(qblk, n)
    nt = n // ktile
    nqb = n // qblk
    qT = q[b].T.astype(npdt)          # [d, n]
    kT = k[b].T.astype(npdt)
    qk = np.empty((d, nqb, 2, qblk), dtype=npdt)
    qk[:, :, 0, :] = kT.reshape(d, nqb, qblk)
    qk[:, :, 1, :] = qT.reshape(d, nqb, qblk)
    v_aug = np.ones((n, d + 1), dtype=np.float32)
    v_aug[:, :d] = v[b]
    v_aug *= (attn_mask[b] != 0).astype(np.float32)[:, None]
    v_aug = np.ascontiguousarray(
        v_aug.reshape(nt, ktile, d + 1).transpose(1, 0, 2)).astype(npdt)
    # triangular 0/1 boundary mask: keep where (q within strip) >= k
    y = np.arange(ktile)[None, :]
    x = np.arange(ktile)[:, None]
    cmasks = (y - x >= 0).astype(npdt)
    return {"qk": qk, "v_aug": v_aug, "cmasks": cmasks}


def kernel(q, k, v, attn_mask):
    global LAST_RESULTS
    q = np.asarray(q, dtype=np.float32)
    k = np.asarray(k, dtype=np.float32)
    v = np.asarray(v, dtype=np.float32)
    attn_mask = np.asarray(attn_mask)

    from concourse.bass_utils import run_bass_kernel_spmd

    nc = _get_nc()
    in_maps = [_prep_core_inputs(q, k, v, attn_mask, b) for b in range(B)]
    trace = bool(os.environ.get("BASS_TRACE"))
    LAST_RESULTS = run_bass_kernel_spmd(
        nc, in_maps, core_ids=list(range(B)), trace=trace)

    out = np.empty((B, N, D), dtype=np.float32)
    for b in range(B):
        out[b] = LAST_RESULTS.results[b]["outT"].T
    return out
